# revision 23
# baseline (speedup 1.0000x reference)
"""Bundle-adjustment projection kernel for 8 Trainium2 NeuronCores.

out[v, n, :] = (u, v) pixel projection of point n under view v
(reference: nn_BundleAdjustmentModel).

Sharding: points N split 8 ways (62500/core); every core computes all 64
views for its slice. On-chip layout: partition p = 64*g + v where g splits
the core's points into 2 halves of 31250 — so every elementwise op runs
128 partitions wide.

The affine work runs on the otherwise-idle PE (tensor engine). Per 512-col
chunk, three bf16 matmuls with block stationaries [39, 128] compute

  a  = (-f*R0 + cx*R2).p + (-f*tx - cx*depth)
  b  = ( f*R1 + cy*R2).p + ( f*ty - cy*depth)
  zc =            R2.p  - depth

zc feeds a pole (clip at |zc| < 1e-4), so plain 16-bit operands are not
accurate enough. Instead both points and coefficients are split 3-way in
bf16 (p = p0+p1+p2, C = C0+C1+C2, ~24 effective mantissa bits) and the
six dominant cross terms are stacked along the matmul K dim — K costs no
cycles (1 col/cycle for any K <= 128), so one bf16 matmul yields a
near-fp32 affine. Moving rows: [p0, p1, p0, p2, p1, p0] x 2 halves (36) +
3 ones rows for a 3-way-split bias. bf16 (not fp16) so no operand ever
goes subnormal.

Tail per chunk, spread so no engine does more than ~2 ops:

  DVE    rc = clip(recip_1nr(zc), +-1e4)   one fused custom-DVE op
         (bitcast-NOT seed + 1 Newton pass + clamp = 7 ALU stages,
         max rel err 1.7e-3; registered into concourse.dve_ops at
         import time)                                    (PSUM->SBUF)
  ACT    pbs = Identity(b)                               (PSUM->SBUF)
  DVE    uv[:, :FW]  = a * rc      tensor_tensor         (PSUM->SBUF)
  GPSIMD uv[:, FW:]  = pbs * rc    tensor_tensor         (SBUF->SBUF;
         GpSimd cannot read PSUM, hence the ACT evacuation; 1x-mode DVE
         never contends with GpSimd on the shared SBUF port pair)
  DMA    uv [128, 1024] fp32 -> HBM (4 KiB/partition contiguous)

uv is block-layout (u-block then v-block per chunk) so every engine
write is stride-1; the host interleaves to [V, N, 2] during unshard.

cx/cy are folded into the PE coefficients (u = (a + cx*zc)/zc = a/zc + cx
exactly when unclipped; error <= cx on clipped points ~ 1.6e-4 of scale).
Host does all O(V) coefficient math + O(N) transposes/splits.
"""
import sys
import types

import numpy as np

V = 64
N = 500000
NC = 8  # cores
N_LOC = N // NC  # 62500 points per core
HALF = N_LOC // 2  # 31250 per partition-half
FW = 512  # chunk width (1 PSUM bank)
NCH = (HALF + FW - 1) // FW  # 62 chunks
F_PAD = NCH * FW  # 31744
K = 39  # moving rows: 6 groups x (3 coords x 2 halves) + 3 ones rows
Z_EPS = 1e-4
RS_MAX = 1.0 / Z_EPS
RC_C0 = -0.23549792  # Chebyshev seed scale (shared with reciprocal_approx_fast)
RC_C1 = 2.0017324
MIN_FOCAL = 50.0
MIN_DISTANCE = 0.25

# term t: sum_t  C[CIDX[t]] . p[PIDX[t]]  (+ 3-way split bias on ones rows)
PIDX = (0, 1, 0, 2, 1, 0)
CIDX = (0, 0, 1, 0, 1, 2)

_CACHE = {}


def _setup_paths():
    if "/opt/trn_rl_repo" not in sys.path:
        sys.path.insert(0, "/opt/trn_rl_repo")
    # the axon trace path imports antenv.axon_hooks; provide a stub if absent
    try:
        import antenv
        if not hasattr(antenv, "axon_hooks"):
            mod = types.ModuleType("antenv.axon_hooks")
            mod._hook = None
            mod.set_axon_ntff_profile_hook = lambda h: setattr(mod, "_hook", h)
            mod.get_axon_ntff_profile_hook = lambda: mod._hook
            sys.modules["antenv.axon_hooks"] = mod
            antenv.axon_hooks = mod
    except ImportError:
        pass


def _recip_clip_op():
    """Fused clip(1/x, +-RS_MAX) as one custom DVE op (7 ALU stages).

    Same bitcast-NOT seed + Chebyshev scale as reciprocal_approx_fast but a
    single Newton pass (max rel err 1.7e-3) to leave stages for the clamp.
    Registered into concourse.dve_ops on first use.
    """
    if "recip_clip" in _CACHE:
        return _CACHE["recip_clip"]
    import numpy as np
    from concourse import dve_ops
    from concourse.dve_spec import AluOp, Bin, C0, C1, C2, Spec, lower, maxx, minn
    from concourse.dve_spec import Src0 as S0
    from concourse.dve_spec import _has_src1 as has_src1
    from concourse.dve_uop import DveOpSpec

    name = "RECIP_CLIP_BA"
    nx = Bin(AluOp.BITWISE_NOT, S0, S0)
    y0 = nx * C0
    y1 = y0 * (C1 - S0 * y0)
    body = minn(maxx(y1, -C2), C2)

    def _ref(in0, in1, c0, c1, c2):
        not_x = (~in0.view(np.int32)).view(np.float32)
        y0 = not_x * np.float32(c0)
        y1 = y0 * (np.float32(c1) - in0 * y0)
        return np.clip(y1, -np.float32(c2), np.float32(c2))

    spec = Spec(body=body, reference=_ref)

    # register the opcode row, then pin the sha by compiling once
    row = dve_ops._CUSTOM_DVE_ROW_BASE + len(dve_ops.OPS)
    dve_ops._SUB_OPCODE_FOR_NAME[name] = row
    shas = {}
    for ver in ("v3", "v4"):
        uops = lower(spec, ver=ver)
        shas[ver] = DveOpSpec(
            name=name, opcode=row, uops=uops, rd1_en=has_src1(spec)
        ).sha(ver)
    op = dve_ops.DveOp(name, spec, subdim=False, uops_sha=shas)
    dve_ops.OPS.append(op)
    dve_ops.CUSTOM_DVE_SPECS[name] = spec
    _CACHE["recip_clip"] = op
    return op


def _build_nc():
    import concourse.bacc as bacc
    import concourse.mybir as mybir
    from concourse import tile

    dt = mybir.dt
    ALU = mybir.AluOpType
    AF = mybir.ActivationFunctionType

    recip_clip = _recip_clip_op()
    nc = bacc.Bacc("TRN2", target_bir_lowering=False, debug=False)
    MOV = nc.dram_tensor("MOV", [K, F_PAD], dt.bfloat16, kind="ExternalInput")
    ST = nc.dram_tensor("ST", [K, 384], dt.bfloat16, kind="ExternalInput")
    OUT = nc.dram_tensor("OUT", [128, 2 * F_PAD], dt.bfloat16,
                         kind="ExternalOutput")

    with tile.TileContext(nc) as tc:
        with (
            tc.tile_pool(name="cst", bufs=1) as cpool,
            tc.tile_pool(name="wrk", bufs=4) as wp,
            tc.tile_pool(name="ps", bufs=1, space="PSUM") as pp,
        ):
            st = cpool.tile([K, 384], dt.bfloat16)
            nc.sync.dma_start(out=st[:], in_=ST.ap())
            # resident moving data; a small first piece so the first pair
            # starts early, bigger pieces stream behind (subtile deps)
            mov = cpool.tile([K, F_PAD], dt.bfloat16)
            pieces = [0, 1024, 4096, 12288, 22016, F_PAD]
            for q in range(len(pieces) - 1):
                nc.sync.dma_start(out=mov[:, pieces[q]:pieces[q + 1]],
                                  in_=MOV.ap()[:, pieces[q]:pieces[q + 1]])

            # pairs of chunks: one LDWEIGHTS per stationary, pair-wide
            # [128, 1024] tail ops, one output DMA. PSUM banks: pz-pair
            # tiles 2x2 + pa/pb chunk tiles 2+2 = 8.
            for p in range(NCH // 2):
                m = mov[:, 2 * p * FW:(2 * p + 2) * FW]
                uv = wp.tile([128, 4 * FW], dt.bfloat16, name="uv", tag="uv",
                             bufs=4)
                pz = pp.tile([128, 2 * FW], dt.float32, name="pz", tag="pz",
                             bufs=1)
                pa = pp.tile([128, 2 * FW], dt.float32, name="pa", tag="pa",
                             bufs=2)
                for h in range(2):
                    nc.tensor.matmul(pz[:, h * FW:(h + 1) * FW],
                                     st[:, 256:384],
                                     m[:, h * FW:(h + 1) * FW],
                                     start=True, stop=True)
                for h in range(2):
                    nc.tensor.matmul(pa[:, h * FW:(h + 1) * FW],
                                     st[:, 0:128],
                                     m[:, h * FW:(h + 1) * FW],
                                     start=True, stop=True)
                pb = pp.tile([128, 2 * FW], dt.float32, name="pb", tag="pb",
                             bufs=1)
                for h in range(2):
                    nc.tensor.matmul(pb[:, h * FW:(h + 1) * FW],
                                     st[:, 128:256],
                                     m[:, h * FW:(h + 1) * FW],
                                     start=True, stop=True)
                rc = wp.tile([128, 2 * FW], dt.float32, name="rc", tag="rc",
                             bufs=4)
                nc.vector._custom_dve(recip_clip, out=rc[:], in0=pz[:],
                                      s0=RC_C0, s1=RC_C1, imm2=RS_MAX)
                pbe = wp.tile([128, 2 * FW], dt.float32, name="pbe", tag="pbe",
                              bufs=4)
                nc.scalar.activation(pbe[:], pb[:], AF.Identity)
                # tt_u reads pa from PSUM (own port) + rc via rd0: stays off
                # the DVE/GpSimd shared SBUF port pair, so the GpSimd
                # tensor_tensor below never blocks on it
                nc.vector.tensor_tensor(uv[:, 0:2 * FW], pa[:], rc[:],
                                        ALU.mult)
                nc.gpsimd.tensor_tensor(uv[:, 2 * FW:4 * FW], pbe[:], rc[:],
                                        ALU.mult)
                nc.sync.dma_start(out=OUT.ap()[:, 4 * p * FW:4 * (p + 1) * FW],
                                  in_=uv)
    nc.compile()
    return nc


def _host_precompute(euler, translation_xy, translation_depth_raw, focal_raw,
                     cx, cy):
    """Per-view coefficient rows (fp32): (Ca, sA), (Cb, sB), (Cz, sZ)."""
    euler = np.asarray(euler, np.float32)
    c = np.cos(euler)
    s = np.sin(euler)
    cx_, cy_, cz_ = c[:, 0], c[:, 1], c[:, 2]
    sx_, sy_, sz_ = s[:, 0], s[:, 1], s[:, 2]
    one = np.ones_like(cx_)
    zero = np.zeros_like(cx_)
    rx = np.stack([
        np.stack([one, zero, zero], -1),
        np.stack([zero, cx_, -sx_], -1),
        np.stack([zero, sx_, cx_], -1)], -2).astype(np.float32)
    ry = np.stack([
        np.stack([cy_, zero, sy_], -1),
        np.stack([zero, one, zero], -1),
        np.stack([-sy_, zero, cy_], -1)], -2).astype(np.float32)
    rz = np.stack([
        np.stack([cz_, -sz_, zero], -1),
        np.stack([sz_, cz_, zero], -1),
        np.stack([zero, zero, one], -1)], -2).astype(np.float32)
    rot = np.matmul(np.matmul(rx, ry), rz).astype(np.float32)  # [V,3,3]

    tdr = np.asarray(translation_depth_raw, np.float32)
    depth = (np.logaddexp(tdr, np.float32(0.0)).astype(np.float32)
             + np.float32(MIN_DISTANCE)).astype(np.float32)
    fr = np.float32(np.asarray(focal_raw).reshape(-1)[0])
    focal = np.float32(np.logaddexp(fr, np.float32(0.0))) + np.float32(MIN_FOCAL)
    txy = np.asarray(translation_xy, np.float32)
    cxf = np.float32(cx)
    cyf = np.float32(cy)

    Ca = -focal * rot[:, 0, :] + cxf * rot[:, 2, :]      # [V,3]
    sA = -focal * txy[:, 0] - cxf * depth                # [V]
    Cb = focal * rot[:, 1, :] + cyf * rot[:, 2, :]
    sB = focal * txy[:, 1] - cyf * depth
    Cz = rot[:, 2, :]
    sZ = -depth
    return (Ca, sA), (Cb, sB), (Cz, sZ)


def _split3(x):
    """3-way bf16 split: x ~ s[0]+s[1]+s[2], each bf16 (as float32)."""
    import ml_dtypes
    x = np.asarray(x, np.float32)
    out = []
    for _ in range(3):
        h = x.astype(ml_dtypes.bfloat16).astype(np.float32)
        out.append(h)
        x = x - h
    return out


def _stationary(C, sbias):
    """[K, 128] fp32 block stationary for one output type."""
    Cs = _split3(C)        # each [V,3]
    ss = _split3(sbias)    # each [V]
    st = np.zeros((K, 128), np.float32)
    for t in range(6):
        Ct = Cs[CIDX[t]]
        for g in range(2):
            cols = slice(64 * g, 64 * g + 64)
            for r in range(3):
                st[6 * t + 3 * g + r, cols] = Ct[:, r]
    for j in range(3):
        st[36 + j, 0:64] = ss[j]
        st[36 + j, 64:128] = ss[j]
    return st


def _moving(sl):
    """[K, F_PAD] fp32 moving block for one core's point slice [62500, 3]."""
    mov = np.zeros((K, F_PAD), np.float32)
    ps = _split3(sl)  # p0, p1, p2 each [62500, 3]
    for t in range(6):
        pt = ps[PIDX[t]]
        for g in range(2):
            seg = pt[g * HALF:(g + 1) * HALF]  # [31250, 3]
            mov[6 * t + 3 * g:6 * t + 3 * g + 3, :HALF] = seg.T
    mov[36:39, :] = 1.0
    return mov


def kernel(points, euler, translation_xy, translation_depth_raw, focal_raw,
           cx, cy, _trace=False):
    _setup_paths()
    import ml_dtypes
    from concourse.bass_utils import run_bass_kernel_spmd

    if "nc" not in _CACHE:
        _CACHE["nc"] = _build_nc()
    nc = _CACHE["nc"]

    points = np.ascontiguousarray(np.asarray(points, np.float32))
    (Ca, sA), (Cb, sB), (Cz, sZ) = _host_precompute(
        euler, translation_xy, translation_depth_raw, focal_raw, cx, cy)

    st = np.concatenate(
        [_stationary(Ca, sA), _stationary(Cb, sB), _stationary(Cz, sZ)],
        axis=1)  # [K, 384]
    st16 = np.ascontiguousarray(st.astype(ml_dtypes.bfloat16))

    in_maps = []
    for k in range(NC):
        sl = points[k * N_LOC:(k + 1) * N_LOC]  # [62500, 3]
        mov16 = np.ascontiguousarray(_moving(sl).astype(ml_dtypes.bfloat16))
        in_maps.append({"MOV": mov16, "ST": st16})

    res = run_bass_kernel_spmd(nc, in_maps, list(range(NC)), trace=_trace)
    _CACHE["last_results"] = res

    out = np.empty((V, N, 2), np.float32)
    for k in range(NC):
        # [128, 2*F_PAD] bf16: per pair p the columns are [u-pair | v-pair]
        o = np.asarray(res.results[k]["OUT"]).astype(np.float32)
        o = o.reshape(128, NCH // 2, 2, 2 * FW)
        for g in range(2):
            seg = np.transpose(o[64 * g:64 * g + 64], (0, 1, 3, 2))
            seg = seg.reshape(64, F_PAD, 2)[:, :HALF, :]
            out[:, k * N_LOC + g * HALF:k * N_LOC + (g + 1) * HALF, :] = seg
    return out


# revision 25
# speedup vs baseline: 1.2136x; 1.2136x over previous
"""Bundle-adjustment projection kernel for 8 Trainium2 NeuronCores.

out[v, n, :] = (u, v) pixel projection of point n under view v
(reference: nn_BundleAdjustmentModel).

Sharding: points N split 8 ways (62500/core); every core computes all 64
views for its slice. On-chip layout: partition p = 64*g + v where g splits
the core's points into 2 halves of 31250 — so every elementwise op runs
128 partitions wide.

The affine work runs on the otherwise-idle PE (tensor engine). Per 512-col
chunk, three bf16 matmuls with block stationaries [39, 128] compute

  a  = (-f*R0 + cx*R2).p + (-f*tx - cx*depth)
  b  = ( f*R1 + cy*R2).p + ( f*ty - cy*depth)
  zc =            R2.p  - depth

zc feeds a pole (clip at |zc| < 1e-4), so plain 16-bit operands are not
accurate enough. Instead both points and coefficients are split 3-way in
bf16 (p = p0+p1+p2, C = C0+C1+C2, ~24 effective mantissa bits) and the
six dominant cross terms are stacked along the matmul K dim — K costs no
cycles (1 col/cycle for any K <= 128), so one bf16 matmul yields a
near-fp32 affine. Moving rows: [p0, p1, p0, p2, p1, p0] x 2 halves (36) +
3 ones rows for a 3-way-split bias. bf16 (not fp16) so no operand ever
goes subnormal.

Tail per chunk, spread so no engine does more than ~2 ops:

  DVE    rc = clip(recip_1nr(zc), +-1e4)   one fused custom-DVE op
         (bitcast-NOT seed + 1 Newton pass + clamp = 7 ALU stages,
         max rel err 1.7e-3; registered into concourse.dve_ops at
         import time)                                    (PSUM->SBUF)
  ACT    pbs = Identity(b)                               (PSUM->SBUF)
  DVE    uv[:, :FW]  = a * rc      tensor_tensor         (PSUM->SBUF)
  GPSIMD uv[:, FW:]  = pbs * rc    tensor_tensor         (SBUF->SBUF;
         GpSimd cannot read PSUM, hence the ACT evacuation; 1x-mode DVE
         never contends with GpSimd on the shared SBUF port pair)
  DMA    uv [128, 1024] fp32 -> HBM (4 KiB/partition contiguous)

uv is block-layout (u-block then v-block per chunk) so every engine
write is stride-1; the host interleaves to [V, N, 2] during unshard.

cx/cy are folded into the PE coefficients (u = (a + cx*zc)/zc = a/zc + cx
exactly when unclipped; error <= cx on clipped points ~ 1.6e-4 of scale).
Host does all O(V) coefficient math + O(N) transposes/splits.
"""
import sys
import types

import numpy as np

V = 64
N = 500000
NC = 8  # cores
N_LOC = N // NC  # 62500 points per core
HALF = N_LOC // 2  # 31250 per partition-half
FW = 512  # chunk width (1 PSUM bank)
NCH = (HALF + FW - 1) // FW  # 62 chunks
F_PAD = NCH * FW  # 31744
K = 39  # moving rows: 6 groups x (3 coords x 2 halves) + 3 ones rows
Z_EPS = 1e-4
RS_MAX = 1.0 / Z_EPS
RC_C0 = -0.23549792  # Chebyshev seed scale (shared with reciprocal_approx_fast)
RC_C1 = 2.0017324
MIN_FOCAL = 50.0
MIN_DISTANCE = 0.25

# term t: sum_t  C[CIDX[t]] . p[PIDX[t]]  (+ 3-way split bias on ones rows)
PIDX = (0, 1, 0, 2, 1, 0)
CIDX = (0, 0, 1, 0, 1, 2)

_CACHE = {}


def _setup_paths():
    if "/opt/trn_rl_repo" not in sys.path:
        sys.path.insert(0, "/opt/trn_rl_repo")
    # the axon trace path imports antenv.axon_hooks; provide a stub if absent
    try:
        import antenv
        if not hasattr(antenv, "axon_hooks"):
            mod = types.ModuleType("antenv.axon_hooks")
            mod._hook = None
            mod.set_axon_ntff_profile_hook = lambda h: setattr(mod, "_hook", h)
            mod.get_axon_ntff_profile_hook = lambda: mod._hook
            sys.modules["antenv.axon_hooks"] = mod
            antenv.axon_hooks = mod
    except ImportError:
        pass


def _recip_clip_op():
    """Fused clip(1/x, +-RS_MAX) as one custom DVE op (7 ALU stages).

    Same bitcast-NOT seed + Chebyshev scale as reciprocal_approx_fast but a
    single Newton pass (max rel err 1.7e-3) to leave stages for the clamp.
    Registered into concourse.dve_ops on first use.
    """
    if "recip_clip" in _CACHE:
        return _CACHE["recip_clip"]
    import numpy as np
    from concourse import dve_ops
    from concourse.dve_spec import AluOp, Bin, C0, C1, C2, Spec, lower, maxx, minn
    from concourse.dve_spec import Src0 as S0
    from concourse.dve_spec import _has_src1 as has_src1
    from concourse.dve_uop import DveOpSpec

    name = "RECIP_CLIP_BA"
    nx = Bin(AluOp.BITWISE_NOT, S0, S0)
    y0 = nx * C0
    y1 = y0 * (C1 - S0 * y0)
    body = minn(maxx(y1, -C2), C2)

    def _ref(in0, in1, c0, c1, c2):
        not_x = (~in0.view(np.int32)).view(np.float32)
        y0 = not_x * np.float32(c0)
        y1 = y0 * (np.float32(c1) - in0 * y0)
        return np.clip(y1, -np.float32(c2), np.float32(c2))

    spec = Spec(body=body, reference=_ref)

    # register the opcode row, then pin the sha by compiling once
    row = dve_ops._CUSTOM_DVE_ROW_BASE + len(dve_ops.OPS)
    dve_ops._SUB_OPCODE_FOR_NAME[name] = row
    shas = {}
    for ver in ("v3", "v4"):
        uops = lower(spec, ver=ver)
        shas[ver] = DveOpSpec(
            name=name, opcode=row, uops=uops, rd1_en=has_src1(spec)
        ).sha(ver)
    op = dve_ops.DveOp(name, spec, subdim=False, uops_sha=shas)
    dve_ops.OPS.append(op)
    dve_ops.CUSTOM_DVE_SPECS[name] = spec
    _CACHE["recip_clip"] = op
    return op


def _build_nc():
    import concourse.bacc as bacc
    import concourse.mybir as mybir
    from concourse import tile

    dt = mybir.dt
    ALU = mybir.AluOpType
    AF = mybir.ActivationFunctionType

    recip_clip = _recip_clip_op()
    nc = bacc.Bacc("TRN2", target_bir_lowering=False, debug=False)
    MOV = nc.dram_tensor("MOV", [K, F_PAD], dt.bfloat16, kind="ExternalInput")
    ST = nc.dram_tensor("ST", [K, 384], dt.bfloat16, kind="ExternalInput")
    OUT = nc.dram_tensor("OUT", [128, 2 * F_PAD], dt.bfloat16,
                         kind="ExternalOutput")

    with tile.TileContext(nc) as tc:
        with (
            tc.tile_pool(name="cst", bufs=1) as cpool,
            tc.tile_pool(name="wrk", bufs=4) as wp,
            tc.tile_pool(name="ps", bufs=1, space="PSUM") as pp,
        ):
            st = cpool.tile([K, 384], dt.bfloat16)
            nc.sync.dma_start(out=st[:], in_=ST.ap())
            # resident moving data; a small first piece so the first pair
            # starts early, bigger pieces stream behind (subtile deps)
            mov = cpool.tile([K, F_PAD], dt.bfloat16)
            pieces = [0, 1024, 4096, 12288, 22016, F_PAD]
            for q in range(len(pieces) - 1):
                nc.sync.dma_start(out=mov[:, pieces[q]:pieces[q + 1]],
                                  in_=MOV.ap()[:, pieces[q]:pieces[q + 1]])

            # pairs of chunks: one LDWEIGHTS per stationary, pair-wide
            # [128, 1024] tail ops, one output DMA. PSUM banks: pz-pair
            # tiles 2x2 + pa/pb chunk tiles 2+2 = 8.
            for p in range(NCH // 2):
                m = mov[:, 2 * p * FW:(2 * p + 2) * FW]
                uv = wp.tile([128, 4 * FW], dt.bfloat16, name="uv", tag="uv",
                             bufs=6)
                pz = pp.tile([128, 2 * FW], dt.float32, name="pz", tag="pz",
                             bufs=1)
                pa = pp.tile([128, 2 * FW], dt.float32, name="pa", tag="pa",
                             bufs=2)
                for h in range(2):
                    nc.tensor.matmul(pz[:, h * FW:(h + 1) * FW],
                                     st[:, 256:384],
                                     m[:, h * FW:(h + 1) * FW],
                                     start=True, stop=True)
                for h in range(2):
                    nc.tensor.matmul(pa[:, h * FW:(h + 1) * FW],
                                     st[:, 0:128],
                                     m[:, h * FW:(h + 1) * FW],
                                     start=True, stop=True)
                pbs_t = []
                for h in range(2):
                    pbs_t.append(pp.tile([128, FW], dt.float32, name="pb",
                                         tag="pb", bufs=2))
                for h in range(2):
                    nc.tensor.matmul(pbs_t[h][:], st[:, 128:256],
                                     m[:, h * FW:(h + 1) * FW],
                                     start=True, stop=True)
                rc = wp.tile([128, 2 * FW], dt.float32, name="rc", tag="rc",
                             bufs=6)
                nc.vector._custom_dve(recip_clip, out=rc[:], in0=pz[:],
                                      s0=RC_C0, s1=RC_C1, imm2=RS_MAX)
                pbe = wp.tile([128, 2 * FW], dt.float32, name="pbe", tag="pbe",
                              bufs=6)
                for h in range(2):
                    nc.scalar.activation(pbe[:, h * FW:(h + 1) * FW],
                                         pbs_t[h][:], AF.Identity)
                # tt_u reads pa from PSUM (own port) + rc via rd0: stays off
                # the DVE/GpSimd shared SBUF port pair, so the GpSimd
                # tensor_tensor below never blocks on it
                nc.vector.tensor_tensor(uv[:, 0:2 * FW], pa[:], rc[:],
                                        ALU.mult)
                nc.gpsimd.tensor_tensor(uv[:, 2 * FW:4 * FW], pbe[:], rc[:],
                                        ALU.mult)
                nc.sync.dma_start(out=OUT.ap()[:, 4 * p * FW:4 * (p + 1) * FW],
                                  in_=uv)
    nc.compile()
    return nc


def _host_precompute(euler, translation_xy, translation_depth_raw, focal_raw,
                     cx, cy):
    """Per-view coefficient rows (fp32): (Ca, sA), (Cb, sB), (Cz, sZ)."""
    euler = np.asarray(euler, np.float32)
    c = np.cos(euler)
    s = np.sin(euler)
    cx_, cy_, cz_ = c[:, 0], c[:, 1], c[:, 2]
    sx_, sy_, sz_ = s[:, 0], s[:, 1], s[:, 2]
    one = np.ones_like(cx_)
    zero = np.zeros_like(cx_)
    rx = np.stack([
        np.stack([one, zero, zero], -1),
        np.stack([zero, cx_, -sx_], -1),
        np.stack([zero, sx_, cx_], -1)], -2).astype(np.float32)
    ry = np.stack([
        np.stack([cy_, zero, sy_], -1),
        np.stack([zero, one, zero], -1),
        np.stack([-sy_, zero, cy_], -1)], -2).astype(np.float32)
    rz = np.stack([
        np.stack([cz_, -sz_, zero], -1),
        np.stack([sz_, cz_, zero], -1),
        np.stack([zero, zero, one], -1)], -2).astype(np.float32)
    rot = np.matmul(np.matmul(rx, ry), rz).astype(np.float32)  # [V,3,3]

    tdr = np.asarray(translation_depth_raw, np.float32)
    depth = (np.logaddexp(tdr, np.float32(0.0)).astype(np.float32)
             + np.float32(MIN_DISTANCE)).astype(np.float32)
    fr = np.float32(np.asarray(focal_raw).reshape(-1)[0])
    focal = np.float32(np.logaddexp(fr, np.float32(0.0))) + np.float32(MIN_FOCAL)
    txy = np.asarray(translation_xy, np.float32)
    cxf = np.float32(cx)
    cyf = np.float32(cy)

    Ca = -focal * rot[:, 0, :] + cxf * rot[:, 2, :]      # [V,3]
    sA = -focal * txy[:, 0] - cxf * depth                # [V]
    Cb = focal * rot[:, 1, :] + cyf * rot[:, 2, :]
    sB = focal * txy[:, 1] - cyf * depth
    Cz = rot[:, 2, :]
    sZ = -depth
    return (Ca, sA), (Cb, sB), (Cz, sZ)


def _split3(x):
    """3-way bf16 split: x ~ s[0]+s[1]+s[2], each bf16 (as float32)."""
    import ml_dtypes
    x = np.asarray(x, np.float32)
    out = []
    for _ in range(3):
        h = x.astype(ml_dtypes.bfloat16).astype(np.float32)
        out.append(h)
        x = x - h
    return out


def _stationary(C, sbias):
    """[K, 128] fp32 block stationary for one output type."""
    Cs = _split3(C)        # each [V,3]
    ss = _split3(sbias)    # each [V]
    st = np.zeros((K, 128), np.float32)
    for t in range(6):
        Ct = Cs[CIDX[t]]
        for g in range(2):
            cols = slice(64 * g, 64 * g + 64)
            for r in range(3):
                st[6 * t + 3 * g + r, cols] = Ct[:, r]
    for j in range(3):
        st[36 + j, 0:64] = ss[j]
        st[36 + j, 64:128] = ss[j]
    return st


def _moving(sl):
    """[K, F_PAD] fp32 moving block for one core's point slice [62500, 3]."""
    mov = np.zeros((K, F_PAD), np.float32)
    ps = _split3(sl)  # p0, p1, p2 each [62500, 3]
    for t in range(6):
        pt = ps[PIDX[t]]
        for g in range(2):
            seg = pt[g * HALF:(g + 1) * HALF]  # [31250, 3]
            mov[6 * t + 3 * g:6 * t + 3 * g + 3, :HALF] = seg.T
    mov[36:39, :] = 1.0
    return mov


def kernel(points, euler, translation_xy, translation_depth_raw, focal_raw,
           cx, cy, _trace=False):
    _setup_paths()
    import ml_dtypes
    from concourse.bass_utils import run_bass_kernel_spmd

    if "nc" not in _CACHE:
        _CACHE["nc"] = _build_nc()
    nc = _CACHE["nc"]

    points = np.ascontiguousarray(np.asarray(points, np.float32))
    (Ca, sA), (Cb, sB), (Cz, sZ) = _host_precompute(
        euler, translation_xy, translation_depth_raw, focal_raw, cx, cy)

    st = np.concatenate(
        [_stationary(Ca, sA), _stationary(Cb, sB), _stationary(Cz, sZ)],
        axis=1)  # [K, 384]
    st16 = np.ascontiguousarray(st.astype(ml_dtypes.bfloat16))

    in_maps = []
    for k in range(NC):
        sl = points[k * N_LOC:(k + 1) * N_LOC]  # [62500, 3]
        mov16 = np.ascontiguousarray(_moving(sl).astype(ml_dtypes.bfloat16))
        in_maps.append({"MOV": mov16, "ST": st16})

    res = run_bass_kernel_spmd(nc, in_maps, list(range(NC)), trace=_trace)
    _CACHE["last_results"] = res

    out = np.empty((V, N, 2), np.float32)
    for k in range(NC):
        # [128, 2*F_PAD] bf16: per pair p the columns are [u-pair | v-pair]
        o = np.asarray(res.results[k]["OUT"]).astype(np.float32)
        o = o.reshape(128, NCH // 2, 2, 2 * FW)
        for g in range(2):
            seg = np.transpose(o[64 * g:64 * g + 64], (0, 1, 3, 2))
            seg = seg.reshape(64, F_PAD, 2)[:, :HALF, :]
            out[:, k * N_LOC + g * HALF:k * N_LOC + (g + 1) * HALF, :] = seg
    return out


# revision 26
# speedup vs baseline: 1.2314x; 1.0147x over previous
"""Bundle-adjustment projection kernel for 8 Trainium2 NeuronCores.

out[v, n, :] = (u, v) pixel projection of point n under view v
(reference: nn_BundleAdjustmentModel).

Sharding: points N split 8 ways (62500/core); every core computes all 64
views for its slice. On-chip layout: partition p = 64*g + v where g splits
the core's points into 2 halves of 31250 — so every elementwise op runs
128 partitions wide.

The affine work runs on the otherwise-idle PE (tensor engine). Per 512-col
chunk, three bf16 matmuls with block stationaries [39, 128] compute

  a  = (-f*R0 + cx*R2).p + (-f*tx - cx*depth)
  b  = ( f*R1 + cy*R2).p + ( f*ty - cy*depth)
  zc =            R2.p  - depth

zc feeds a pole (clip at |zc| < 1e-4), so plain 16-bit operands are not
accurate enough. Instead both points and coefficients are split 3-way in
bf16 (p = p0+p1+p2, C = C0+C1+C2, ~24 effective mantissa bits) and the
six dominant cross terms are stacked along the matmul K dim — K costs no
cycles (1 col/cycle for any K <= 128), so one bf16 matmul yields a
near-fp32 affine. Moving rows: [p0, p1, p0, p2, p1, p0] x 2 halves (36) +
3 ones rows for a 3-way-split bias. bf16 (not fp16) so no operand ever
goes subnormal.

Tail per chunk, spread so no engine does more than ~2 ops:

  DVE    rc = clip(recip_1nr(zc), +-1e4)   one fused custom-DVE op
         (bitcast-NOT seed + 1 Newton pass + clamp = 7 ALU stages,
         max rel err 1.7e-3; registered into concourse.dve_ops at
         import time)                                    (PSUM->SBUF)
  ACT    pbs = Identity(b)                               (PSUM->SBUF)
  DVE    uv[:, :FW]  = a * rc      tensor_tensor         (PSUM->SBUF)
  GPSIMD uv[:, FW:]  = pbs * rc    tensor_tensor         (SBUF->SBUF;
         GpSimd cannot read PSUM, hence the ACT evacuation; 1x-mode DVE
         never contends with GpSimd on the shared SBUF port pair)
  DMA    uv [128, 1024] fp32 -> HBM (4 KiB/partition contiguous)

uv is block-layout (u-block then v-block per chunk) so every engine
write is stride-1; the host interleaves to [V, N, 2] during unshard.

cx/cy are folded into the PE coefficients (u = (a + cx*zc)/zc = a/zc + cx
exactly when unclipped; error <= cx on clipped points ~ 1.6e-4 of scale).
Host does all O(V) coefficient math + O(N) transposes/splits.
"""
import sys
import types

import numpy as np

V = 64
N = 500000
NC = 8  # cores
N_LOC = N // NC  # 62500 points per core
HALF = N_LOC // 2  # 31250 per partition-half
FW = 512  # chunk width (1 PSUM bank)
NCH = (HALF + FW - 1) // FW  # 62 chunks
F_PAD = NCH * FW  # 31744
K = 39  # moving rows: 6 groups x (3 coords x 2 halves) + 3 ones rows
Z_EPS = 1e-4
RS_MAX = 1.0 / Z_EPS
RC_C0 = -0.23549792  # Chebyshev seed scale (shared with reciprocal_approx_fast)
RC_C1 = 2.0017324
MIN_FOCAL = 50.0
MIN_DISTANCE = 0.25

# term t: sum_t  C[CIDX[t]] . p[PIDX[t]]  (+ 3-way split bias on ones rows)
PIDX = (0, 1, 0, 2, 1, 0)
CIDX = (0, 0, 1, 0, 1, 2)

_CACHE = {}


def _setup_paths():
    if "/opt/trn_rl_repo" not in sys.path:
        sys.path.insert(0, "/opt/trn_rl_repo")
    # the axon trace path imports antenv.axon_hooks; provide a stub if absent
    try:
        import antenv
        if not hasattr(antenv, "axon_hooks"):
            mod = types.ModuleType("antenv.axon_hooks")
            mod._hook = None
            mod.set_axon_ntff_profile_hook = lambda h: setattr(mod, "_hook", h)
            mod.get_axon_ntff_profile_hook = lambda: mod._hook
            sys.modules["antenv.axon_hooks"] = mod
            antenv.axon_hooks = mod
    except ImportError:
        pass


def _recip_clip_op():
    """Fused clip(1/x, +-RS_MAX) as one custom DVE op (7 ALU stages).

    Same bitcast-NOT seed + Chebyshev scale as reciprocal_approx_fast but a
    single Newton pass (max rel err 1.7e-3) to leave stages for the clamp.
    Registered into concourse.dve_ops on first use.
    """
    if "recip_clip" in _CACHE:
        return _CACHE["recip_clip"]
    import numpy as np
    from concourse import dve_ops
    from concourse.dve_spec import AluOp, Bin, C0, C1, C2, Spec, lower, maxx, minn
    from concourse.dve_spec import Src0 as S0
    from concourse.dve_spec import _has_src1 as has_src1
    from concourse.dve_uop import DveOpSpec

    name = "RECIP_CLIP_BA"
    nx = Bin(AluOp.BITWISE_NOT, S0, S0)
    y0 = nx * C0
    y1 = y0 * (C1 - S0 * y0)
    body = minn(maxx(y1, -C2), C2)

    def _ref(in0, in1, c0, c1, c2):
        not_x = (~in0.view(np.int32)).view(np.float32)
        y0 = not_x * np.float32(c0)
        y1 = y0 * (np.float32(c1) - in0 * y0)
        return np.clip(y1, -np.float32(c2), np.float32(c2))

    spec = Spec(body=body, reference=_ref)

    # register the opcode row, then pin the sha by compiling once
    row = dve_ops._CUSTOM_DVE_ROW_BASE + len(dve_ops.OPS)
    dve_ops._SUB_OPCODE_FOR_NAME[name] = row
    shas = {}
    for ver in ("v3", "v4"):
        uops = lower(spec, ver=ver)
        shas[ver] = DveOpSpec(
            name=name, opcode=row, uops=uops, rd1_en=has_src1(spec)
        ).sha(ver)
    op = dve_ops.DveOp(name, spec, subdim=False, uops_sha=shas)
    dve_ops.OPS.append(op)
    dve_ops.CUSTOM_DVE_SPECS[name] = spec
    _CACHE["recip_clip"] = op
    return op


def _build_nc():
    import concourse.bacc as bacc
    import concourse.mybir as mybir
    from concourse import tile

    dt = mybir.dt
    ALU = mybir.AluOpType
    AF = mybir.ActivationFunctionType

    recip_clip = _recip_clip_op()
    nc = bacc.Bacc("TRN2", target_bir_lowering=False, debug=False)
    MOV = nc.dram_tensor("MOV", [K, F_PAD], dt.bfloat16, kind="ExternalInput")
    ST = nc.dram_tensor("ST", [K, 384], dt.bfloat16, kind="ExternalInput")
    OUT = nc.dram_tensor("OUT", [128, 2 * F_PAD], dt.bfloat16,
                         kind="ExternalOutput")

    with tile.TileContext(nc) as tc:
        with (
            tc.tile_pool(name="cst", bufs=1) as cpool,
            tc.tile_pool(name="wrk", bufs=4) as wp,
            tc.tile_pool(name="ps", bufs=1, space="PSUM") as pp,
        ):
            st = cpool.tile([K, 384], dt.bfloat16)
            nc.sync.dma_start(out=st[:], in_=ST.ap())
            # resident moving data; a small first piece so the first pair
            # starts early, bigger pieces stream behind (subtile deps)
            mov = cpool.tile([K, F_PAD], dt.bfloat16)
            pieces = [0, 1024, 4096, 12288, 22016, F_PAD]
            for q in range(len(pieces) - 1):
                nc.sync.dma_start(out=mov[:, pieces[q]:pieces[q + 1]],
                                  in_=MOV.ap()[:, pieces[q]:pieces[q + 1]])

            # pairs of chunks: one LDWEIGHTS per stationary, pair-wide
            # [128, 1024] tail ops, one output DMA. PSUM banks: pz-pair
            # 1x2 + pa-pair 2x2 + pb chunk tiles 2 = 8. The multiply tail
            # of pair p is emitted AFTER pair p+1's matmuls+recip, so on
            # DVE's strict FIFO the recip (which frees the single pz pair
            # and unblocks PE) always runs before the previous tt_u.
            def emit_mms_recip(p):
                m = mov[:, 2 * p * FW:(2 * p + 2) * FW]
                pz = pp.tile([128, 2 * FW], dt.float32, name="pz", tag="pz",
                             bufs=1)
                pa = pp.tile([128, 2 * FW], dt.float32, name="pa", tag="pa",
                             bufs=2)
                for h in range(2):
                    nc.tensor.matmul(pz[:, h * FW:(h + 1) * FW],
                                     st[:, 256:384],
                                     m[:, h * FW:(h + 1) * FW],
                                     start=True, stop=True)
                for h in range(2):
                    nc.tensor.matmul(pa[:, h * FW:(h + 1) * FW],
                                     st[:, 0:128],
                                     m[:, h * FW:(h + 1) * FW],
                                     start=True, stop=True)
                pbs_t = []
                for h in range(2):
                    pbs_t.append(pp.tile([128, FW], dt.float32, name="pb",
                                         tag="pb", bufs=2))
                for h in range(2):
                    nc.tensor.matmul(pbs_t[h][:], st[:, 128:256],
                                     m[:, h * FW:(h + 1) * FW],
                                     start=True, stop=True)
                rc = wp.tile([128, 2 * FW], dt.float32, name="rc", tag="rc",
                             bufs=6)
                nc.vector._custom_dve(recip_clip, out=rc[:], in0=pz[:],
                                      s0=RC_C0, s1=RC_C1, imm2=RS_MAX)
                pbe = wp.tile([128, 2 * FW], dt.float32, name="pbe", tag="pbe",
                              bufs=6)
                for h in range(2):
                    nc.scalar.activation(pbe[:, h * FW:(h + 1) * FW],
                                         pbs_t[h][:], AF.Identity)
                return pa, rc, pbe

            def emit_tail(p, pa, rc, pbe):
                uv = wp.tile([128, 4 * FW], dt.bfloat16, name="uv", tag="uv",
                             bufs=6)
                # tt_u reads pa from PSUM (own port) + rc via rd0: stays off
                # the DVE/GpSimd shared SBUF port pair, so the GpSimd
                # tensor_tensor below never blocks on it
                nc.vector.tensor_tensor(uv[:, 0:2 * FW], pa[:], rc[:],
                                        ALU.mult)
                nc.gpsimd.tensor_tensor(uv[:, 2 * FW:4 * FW], pbe[:], rc[:],
                                        ALU.mult)
                nc.sync.dma_start(out=OUT.ap()[:, 4 * p * FW:4 * (p + 1) * FW],
                                  in_=uv)

            pending = None
            for p in range(NCH // 2):
                cur = emit_mms_recip(p)
                if pending is not None:
                    emit_tail(p - 1, *pending)
                pending = cur
            emit_tail(NCH // 2 - 1, *pending)
    nc.compile()
    return nc


def _host_precompute(euler, translation_xy, translation_depth_raw, focal_raw,
                     cx, cy):
    """Per-view coefficient rows (fp32): (Ca, sA), (Cb, sB), (Cz, sZ)."""
    euler = np.asarray(euler, np.float32)
    c = np.cos(euler)
    s = np.sin(euler)
    cx_, cy_, cz_ = c[:, 0], c[:, 1], c[:, 2]
    sx_, sy_, sz_ = s[:, 0], s[:, 1], s[:, 2]
    one = np.ones_like(cx_)
    zero = np.zeros_like(cx_)
    rx = np.stack([
        np.stack([one, zero, zero], -1),
        np.stack([zero, cx_, -sx_], -1),
        np.stack([zero, sx_, cx_], -1)], -2).astype(np.float32)
    ry = np.stack([
        np.stack([cy_, zero, sy_], -1),
        np.stack([zero, one, zero], -1),
        np.stack([-sy_, zero, cy_], -1)], -2).astype(np.float32)
    rz = np.stack([
        np.stack([cz_, -sz_, zero], -1),
        np.stack([sz_, cz_, zero], -1),
        np.stack([zero, zero, one], -1)], -2).astype(np.float32)
    rot = np.matmul(np.matmul(rx, ry), rz).astype(np.float32)  # [V,3,3]

    tdr = np.asarray(translation_depth_raw, np.float32)
    depth = (np.logaddexp(tdr, np.float32(0.0)).astype(np.float32)
             + np.float32(MIN_DISTANCE)).astype(np.float32)
    fr = np.float32(np.asarray(focal_raw).reshape(-1)[0])
    focal = np.float32(np.logaddexp(fr, np.float32(0.0))) + np.float32(MIN_FOCAL)
    txy = np.asarray(translation_xy, np.float32)
    cxf = np.float32(cx)
    cyf = np.float32(cy)

    Ca = -focal * rot[:, 0, :] + cxf * rot[:, 2, :]      # [V,3]
    sA = -focal * txy[:, 0] - cxf * depth                # [V]
    Cb = focal * rot[:, 1, :] + cyf * rot[:, 2, :]
    sB = focal * txy[:, 1] - cyf * depth
    Cz = rot[:, 2, :]
    sZ = -depth
    return (Ca, sA), (Cb, sB), (Cz, sZ)


def _split3(x):
    """3-way bf16 split: x ~ s[0]+s[1]+s[2], each bf16 (as float32)."""
    import ml_dtypes
    x = np.asarray(x, np.float32)
    out = []
    for _ in range(3):
        h = x.astype(ml_dtypes.bfloat16).astype(np.float32)
        out.append(h)
        x = x - h
    return out


def _stationary(C, sbias):
    """[K, 128] fp32 block stationary for one output type."""
    Cs = _split3(C)        # each [V,3]
    ss = _split3(sbias)    # each [V]
    st = np.zeros((K, 128), np.float32)
    for t in range(6):
        Ct = Cs[CIDX[t]]
        for g in range(2):
            cols = slice(64 * g, 64 * g + 64)
            for r in range(3):
                st[6 * t + 3 * g + r, cols] = Ct[:, r]
    for j in range(3):
        st[36 + j, 0:64] = ss[j]
        st[36 + j, 64:128] = ss[j]
    return st


def _moving(sl):
    """[K, F_PAD] fp32 moving block for one core's point slice [62500, 3]."""
    mov = np.zeros((K, F_PAD), np.float32)
    ps = _split3(sl)  # p0, p1, p2 each [62500, 3]
    for t in range(6):
        pt = ps[PIDX[t]]
        for g in range(2):
            seg = pt[g * HALF:(g + 1) * HALF]  # [31250, 3]
            mov[6 * t + 3 * g:6 * t + 3 * g + 3, :HALF] = seg.T
    mov[36:39, :] = 1.0
    return mov


def kernel(points, euler, translation_xy, translation_depth_raw, focal_raw,
           cx, cy, _trace=False):
    _setup_paths()
    import ml_dtypes
    from concourse.bass_utils import run_bass_kernel_spmd

    if "nc" not in _CACHE:
        _CACHE["nc"] = _build_nc()
    nc = _CACHE["nc"]

    points = np.ascontiguousarray(np.asarray(points, np.float32))
    (Ca, sA), (Cb, sB), (Cz, sZ) = _host_precompute(
        euler, translation_xy, translation_depth_raw, focal_raw, cx, cy)

    st = np.concatenate(
        [_stationary(Ca, sA), _stationary(Cb, sB), _stationary(Cz, sZ)],
        axis=1)  # [K, 384]
    st16 = np.ascontiguousarray(st.astype(ml_dtypes.bfloat16))

    in_maps = []
    for k in range(NC):
        sl = points[k * N_LOC:(k + 1) * N_LOC]  # [62500, 3]
        mov16 = np.ascontiguousarray(_moving(sl).astype(ml_dtypes.bfloat16))
        in_maps.append({"MOV": mov16, "ST": st16})

    res = run_bass_kernel_spmd(nc, in_maps, list(range(NC)), trace=_trace)
    _CACHE["last_results"] = res

    out = np.empty((V, N, 2), np.float32)
    for k in range(NC):
        # [128, 2*F_PAD] bf16: per pair p the columns are [u-pair | v-pair]
        o = np.asarray(res.results[k]["OUT"]).astype(np.float32)
        o = o.reshape(128, NCH // 2, 2, 2 * FW)
        for g in range(2):
            seg = np.transpose(o[64 * g:64 * g + 64], (0, 1, 3, 2))
            seg = seg.reshape(64, F_PAD, 2)[:, :HALF, :]
            out[:, k * N_LOC + g * HALF:k * N_LOC + (g + 1) * HALF, :] = seg
    return out
